# revision 4
# baseline (speedup 1.0000x reference)
"""Trainium2 Bass kernel for nn_PixelCrossAttentionRefiner.

Semantics (matching the reference's torch-style reshape): per batch b, the
flat candidate stream m = k*N + n2 (K=8 slices of N=H*W pixels) is chunked
into groups of 8 consecutive entries; query pixel n attends over entries
8n..8n+7. Key/value entries carry coords (and stage-2 z) of their OWN flat
position n2 = m mod N.

Sharding: 8 cores = 4 batches x 2 halves. Core c: batch b=c//2, half
h=c%2 -> candidate k-slices [4h, 4h+4), queries n in [8192h, 8192h+8192).
Stage-2 keys need z at all N positions of the batch, so z is exchanged on
the host between the two per-batch kernel launches (stage1, stage2).

Layout on device: E-major ([feature, pixel]) tiles; candidate tiles
[128, 512] stack two 512-column ranges of one k-slice (rows 0:64 | 64:128).
Per tile: projections on PE (fp32r / fp16), per-(query,head) scores via an
indicator matmul over partitions, softmax on 8-row tiles, attn expansion
back to [128, 512] via a second indicator matmul, weighted sum + output
projection.
"""

import numpy as np

import concourse.bass as bass
import concourse.mybir as mybir
import concourse.tile as tile
from concourse import bacc
from concourse.bass_utils import run_bass_kernel_spmd

F32 = mybir.dt.float32
F32R = mybir.dt.float32r
F16 = mybir.dt.float16

B, C, H, W = 4, 64, 128, 128
K, E, NH = 8, 64, 4
N = H * W            # 16384 pixels per batch
NCORES = 8
SLICES = 4           # k-slices per core
QPC = N // 2         # queries per core (8192)
TF = 512             # free width per half-tile
TC = 2 * TF          # candidate columns per tile (1024)
NTS = N // TC        # tiles per slice (16)
AX = mybir.AxisListType
ALU = mybir.AluOpType
ACTF = mybir.ActivationFunctionType


# ----------------------------------------------------------------------------
# host-side weight fusion
# ----------------------------------------------------------------------------

def _fuse_weights(inp):
    wq, wk, wv = np.split(inp["in_proj_w"], 3, axis=0)
    bq, bk, bv = np.split(inp["in_proj_b"], 3, axis=0)
    out_w, out_b = inp["out_w"], inp["out_b"]

    Wq1f = inp["q1_w"].T @ wq.T
    bq1f = inp["q1_b"] @ wq.T + bq
    Wk1f = inp["k1_w"].T @ wk.T
    Wv1f = inp["v1_w"].T @ wv.T
    bv1f = inp["v1_b"] @ wv.T + bv
    zb1 = bv1f @ out_w.T + out_b

    Wq2f = inp["q2_w"].T @ wq.T
    bq2f = inp["q2_b"] @ wq.T + bq
    Wk2f = inp["k2_w"].T @ wk.T
    Wv2f = inp["v2_w"].T @ wv.T
    bv2f = inp["v2_b"] @ wv.T + bv
    zb2 = bv2f @ out_w.T + out_b

    def blk(a):  # [64, 64] -> [128, 128] block diagonal
        o = np.zeros((128, 128), a.dtype)
        o[:64, :64] = a
        o[64:, 64:] = a
        return o

    def ablk(rows):  # [4, 64] affine rows -> [8, 128] block diagonal
        o = np.zeros((8, 128), np.float32)
        o[:4, :64] = rows
        o[4:, 64:] = rows
        return o

    z4 = np.zeros((1, E), np.float32)
    wgt = {
        # stage 1
        "wq1": np.concatenate([Wq1f[:64], Wq1f[64:66], bq1f[None], z4], 0).astype(np.float16),
        "wk1x": blk(Wk1f[:64]).astype(np.float32),
        "wk1a": ablk(np.concatenate([Wk1f[64:66], z4, z4], 0)),
        "wv1": blk(Wv1f).astype(np.float32),
        "zb1": np.tile(zb1, 2).astype(np.float32).reshape(128, 1),
        # stage 2
        "wq2": np.concatenate([Wq2f[:64], Wq2f[128:130], bq2f[None], z4], 0).astype(np.float16),
        "wq2z": Wq2f[64:128].astype(np.float16),
        "wk2x": blk(Wk2f[:64]).astype(np.float32),
        "wk2z": blk(Wk2f[64:128]).astype(np.float16),
        "wk2a": ablk(np.concatenate([Wk2f[128:130], z4, z4], 0)),
        "wv2x": blk(Wv2f[:64]).astype(np.float32),
        "wv2z": blk(Wv2f[64:128]).astype(np.float16),
        "zb2": np.tile(zb2, 2).astype(np.float32).reshape(128, 1),
        # shared
        "wo": blk(out_w.T).astype(np.float32),
    }

    # score indicator: partition (64*half + 16h + d) -> col (4*half + h)
    ind_s = np.zeros((128, 8), np.float16)
    for p in range(128):
        ind_s[p, 4 * (p // 64) + (p % 64) // 16] = 1.0
    wgt["ind_s"] = ind_s
    # expansion indicator: row (4*half + h) -> partitions 64*half + 16h + d
    wgt["ind_e"] = ind_s.T.copy()
    return wgt


def _affine_tables():
    i = np.linspace(0.0, 1.0, H, dtype=np.float32)
    j = np.linspace(0.0, 1.0, W, dtype=np.float32)
    ig, jg = np.meshgrid(i, j, indexing="ij")
    aff = np.stack([ig.ravel(), jg.ravel(),
                    np.ones(N, np.float32), np.zeros(N, np.float32)])  # [4, N]
    return aff


# ----------------------------------------------------------------------------
# bass program (one stage)
# ----------------------------------------------------------------------------

WSPECS1 = [("wq1", (68, 64), F16), ("wk1x", (128, 128), F32R),
           ("wk1a", (8, 128), F32R), ("wv1", (128, 128), F32R),
           ("zb1", (128, 1), F32),
           ("ind_s", (128, 8), F16), ("ind_e", (8, 128), F16),
           ("wo", (128, 128), F32)]
WSPECS2 = [("wq2", (68, 64), F16), ("wq2z", (64, 64), F16),
           ("wk2x", (128, 128), F32R), ("wk2z", (128, 128), F16),
           ("wk2a", (8, 128), F32R), ("wv2x", (128, 128), F32R),
           ("wv2z", (128, 128), F16), ("zb2", (128, 1), F32),
           ("ind_s", (128, 8), F16), ("ind_e", (8, 128), F16),
           ("wo", (128, 128), F32)]


def build_program(stage: int):
    nc = bacc.Bacc("TRN2", target_bir_lowering=False, debug=False)

    kc = nc.dram_tensor("kc", [SLICES, C, N], F32R, kind="ExternalInput")
    vc = nc.dram_tensor("vc", [SLICES, C, N], F32R, kind="ExternalInput")
    xq = nc.dram_tensor("xq", [C, QPC], F32, kind="ExternalInput")
    affq = nc.dram_tensor("affq", [4, QPC], F32, kind="ExternalInput")
    affc = nc.dram_tensor("affc", [4, N], F32R, kind="ExternalInput")
    wspecs = WSPECS1 if stage == 1 else WSPECS2
    wd = {nm: nc.dram_tensor(nm, list(sh), dt, kind="ExternalInput")
          for nm, sh, dt in wspecs}
    if stage == 2:
        zfull = nc.dram_tensor("zfull", [C, N], F16, kind="ExternalInput")
        zq = nc.dram_tensor("zq", [C, QPC], F16, kind="ExternalInput")
        out = nc.dram_tensor("yout", [C, QPC], F32, kind="ExternalOutput")
    else:
        out = nc.dram_tensor("zout", [C, QPC], F16, kind="ExternalOutput")

    with tile.TileContext(nc) as tc:
        with (
            tc.tile_pool(name="wpool", bufs=1) as wpool,
            tc.tile_pool(name="io", bufs=3) as io,
            tc.tile_pool(name="aux", bufs=2) as aux,
            tc.tile_pool(name="mid", bufs=2) as mid,
            tc.tile_pool(name="ps_kh", bufs=2, space="PSUM") as ps_kh,
            tc.tile_pool(name="ps_vh", bufs=2, space="PSUM") as ps_vh,
            tc.tile_pool(name="ps_ae", bufs=1, space="PSUM") as ps_ae,
            tc.tile_pool(name="ps_small", bufs=1, space="PSUM") as ps_small,
        ):
            # load weights once
            wt = {}
            for nm, sh, dt in wspecs:
                t = wpool.tile(list(sh), dt, tag=nm)
                nc.sync.dma_start(t[:], wd[nm].ap())
                wt[nm] = t

            for s in range(SLICES):
                for ti in range(NTS):
                    t0 = ti * TC
                    q0 = s * (N // K) + ti * (TC // K)  # local query offset

                    kc_t = io.tile([128, TF], F32R, tag="kc_t")
                    nc.sync.dma_start(kc_t[0:64, :], kc[s, :, t0:t0 + TF])
                    nc.sync.dma_start(kc_t[64:128, :], kc[s, :, t0 + TF:t0 + TC])
                    vc_t = io.tile([128, TF], F32R, tag="vc_t")
                    nc.sync.dma_start(vc_t[0:64, :], vc[s, :, t0:t0 + TF])
                    nc.sync.dma_start(vc_t[64:128, :], vc[s, :, t0 + TF:t0 + TC])
                    aff_t = aux.tile([8, TF], F32R, tag="aff_t")
                    nc.sync.dma_start(aff_t[0:4, :], affc[:, t0:t0 + TF])
                    nc.sync.dma_start(aff_t[4:8, :], affc[:, t0 + TF:t0 + TC])
                    xa_t = aux.tile([68, 128], F32, tag="xa_t")
                    nc.sync.dma_start(xa_t[0:64, :], xq[:, q0:q0 + 128])
                    nc.sync.dma_start(xa_t[64:68, :], affq[:, q0:q0 + 128])
                    xa16 = aux.tile([68, 128], F16, tag="xa16")
                    nc.scalar.copy(xa16[:], xa_t[:])
                    if stage == 2:
                        zq_t = aux.tile([64, 128], F16, tag="zq_t")
                        nc.sync.dma_start(zq_t[:], zq[:, q0:q0 + 128])
                        zst_t = io.tile([128, TF], F16, tag="zst_t")
                        nc.sync.dma_start(zst_t[0:64, :], zfull[:, t0:t0 + TF])
                        nc.sync.dma_start(zst_t[64:128, :], zfull[:, t0 + TF:t0 + TC])

                    # ---- query projection: qh [128, 64] (top/bottom query halves)
                    qh_ps = ps_small.tile([128, 64], F32, tag="qh_ps")
                    wq = wt["wq1" if stage == 1 else "wq2"]
                    two = stage == 2
                    nc.tensor.matmul(qh_ps[0:64, :], wq[:], xa16[:, 0:64],
                                     start=True, stop=not two)
                    nc.tensor.matmul(qh_ps[64:128, :], wq[:], xa16[:, 64:128],
                                     start=True, stop=not two)
                    if two:
                        nc.tensor.matmul(qh_ps[0:64, :], wt["wq2z"][:],
                                         zq_t[:, 0:64], start=False, stop=True)
                        nc.tensor.matmul(qh_ps[64:128, :], wt["wq2z"][:],
                                         zq_t[:, 64:128], start=False, stop=True)
                    qh16 = mid.tile([128, 64], F16, tag="qh16")
                    nc.vector.tensor_copy(qh16[:], qh_ps[:])

                    # ---- key projection: kh [128, TF]
                    kh_ps = ps_kh.tile([128, TF], F32, tag="kh_ps")
                    if stage == 1:
                        nc.tensor.matmul(kh_ps[:], wt["wk1x"][:],
                                         kc_t[:], start=True, stop=False)
                        nc.tensor.matmul(kh_ps[:], wt["wk1a"][:],
                                         aff_t[:], start=False, stop=True)
                    else:
                        nc.tensor.matmul(kh_ps[:], wt["wk2x"][:],
                                         kc_t[:], start=True, stop=False)
                        nc.tensor.matmul(kh_ps[:], wt["wk2a"][:],
                                         aff_t[:], start=False, stop=False)
                        nc.tensor.matmul(kh_ps[:], wt["wk2z"][:], zst_t[:],
                                         start=False, stop=True)
                    kh16 = mid.tile([128, TF], F16, tag="kh16")
                    nc.scalar.copy(kh16[:], kh_ps[:])

                    # ---- scores: prods then indicator matmul over partitions
                    p16 = mid.tile([128, TF], F16, tag="p16")
                    nc.vector.tensor_mul(
                        p16[:].rearrange("p (q k) -> p q k", k=K),
                        kh16[:].rearrange("p (q k) -> p q k", k=K),
                        qh16[:].broadcast_to([128, 64, K]))
                    sc_ps = ps_small.tile([8, TF], F32, tag="sc_ps")
                    nc.tensor.matmul(sc_ps[:], wt["ind_s"][:], p16[:],
                                     start=True, stop=True)

                    # ---- softmax over k (8 consecutive columns per query)
                    e16 = mid.tile([8, TF], F16, tag="e16")
                    nc.scalar.activation(e16[:], sc_ps[:], ACTF.Exp, scale=0.25)
                    den = aux.tile([8, 64], F32, tag="den")
                    nc.vector.tensor_reduce(
                        den[:], e16[:].rearrange("p (q k) -> p q k", k=K),
                        axis=AX.X, op=ALU.add)
                    rcp = aux.tile([8, 64], F32, tag="rcp")
                    nc.vector.reciprocal(rcp[:], den[:])
                    at16 = mid.tile([8, TF], F16, tag="at16")
                    nc.vector.tensor_mul(
                        at16[:].rearrange("p (q k) -> p q k", k=K),
                        e16[:].rearrange("p (q k) -> p q k", k=K),
                        rcp[:].broadcast_to([8, 64, K]))

                    # ---- expand attn to [128, TF]
                    ae_ps = ps_ae.tile([128, TF], F32, tag="ae_ps")
                    nc.tensor.matmul(ae_ps[:], wt["ind_e"][:], at16[:],
                                     start=True, stop=True)
                    ae16 = mid.tile([128, TF], F16, tag="ae16")
                    nc.scalar.copy(ae16[:], ae_ps[:])

                    # ---- value projection
                    vh_ps = ps_vh.tile([128, TF], F32, tag="vh_ps")
                    if stage == 1:
                        nc.tensor.matmul(vh_ps[:], wt["wv1"][:],
                                         vc_t[:], start=True, stop=True)
                    else:
                        nc.tensor.matmul(vh_ps[:], wt["wv2x"][:],
                                         vc_t[:], start=True, stop=False)
                        nc.tensor.matmul(vh_ps[:], wt["wv2z"][:], zst_t[:],
                                         start=False, stop=True)

                    # ---- weighted values, summed over each query's 8 entries
                    w16 = mid.tile([128, TF], F16, tag="w16")
                    nc.vector.tensor_mul(w16[:], vh_ps[:], ae16[:])
                    o_t = mid.tile([128, 64], F32, tag="o_t")
                    nc.vector.tensor_reduce(
                        o_t[:], w16[:].rearrange("p (q k) -> p q k", k=K),
                        axis=AX.X, op=ALU.add)

                    # ---- output projection + bias
                    z_ps = ps_small.tile([128, 64], F32, tag="z_ps")
                    nc.tensor.matmul(z_ps[:], wt["wo"][:], o_t[:],
                                     start=True, stop=True)
                    zb = wt["zb1" if stage == 1 else "zb2"]
                    odt = F16 if stage == 1 else F32
                    z_sb = mid.tile([128, 64], odt, tag="z_sb")
                    nc.scalar.activation(z_sb[:], z_ps[:], ACTF.Identity,
                                         bias=zb[:, 0:1])
                    nc.sync.dma_start(out.ap()[:, q0:q0 + 64], z_sb[0:64, :])
                    nc.sync.dma_start(out.ap()[:, q0 + 64:q0 + 128], z_sb[64:128, :])

    nc.compile()
    return nc


# ----------------------------------------------------------------------------
# host orchestration
# ----------------------------------------------------------------------------

_progs = {}


def _prog(stage):
    if stage not in _progs:
        _progs[stage] = build_program(stage)
    return _progs[stage]


def kernel(**inputs):
    inputs = {k: np.ascontiguousarray(np.asarray(v)) for k, v in inputs.items()}
    wgt = _fuse_weights(inputs)
    aff = _affine_tables()

    hfc = inputs["HF_cands"]   # [B, K, C, H, W]
    zc = inputs["Z_cands"]
    hfs = inputs["HF_star"]    # [B, C, H, W]

    wmap1 = {nm: wgt[nm] for nm, _, _ in WSPECS1}
    wmap2 = {nm: wgt[nm] for nm, _, _ in WSPECS2}

    def core_base(c):
        b, h = divmod(c, 2)
        sl = slice(4 * h, 4 * h + 4)
        qs = slice(QPC * h, QPC * h + QPC)
        return {
            "kc": hfc[b, sl].reshape(SLICES, C, N),
            "vc": zc[b, sl].reshape(SLICES, C, N),
            "xq": hfs[b].reshape(C, N)[:, qs],
            "affq": aff[:, qs],
            "affc": aff,
        }

    bases = [core_base(c) for c in range(NCORES)]

    in1 = [{**bases[c], **wmap1} for c in range(NCORES)]
    r1 = run_bass_kernel_spmd(_prog(1), in1, core_ids=list(range(NCORES)))
    z_half = [r1.results[c]["zout"] for c in range(NCORES)]  # [64, 8192] f16

    in2 = []
    for c in range(NCORES):
        b, h = divmod(c, 2)
        zf = np.concatenate([z_half[2 * b], z_half[2 * b + 1]], axis=1)  # [64, N]
        in2.append({**bases[c], **wmap2, "zfull": zf, "zq": z_half[c]})
    r2 = run_bass_kernel_spmd(_prog(2), in2, core_ids=list(range(NCORES)))

    y = np.empty((B, C, H, W), np.float32)
    for c in range(NCORES):
        b, h = divmod(c, 2)
        y[b].reshape(C, N)[:, QPC * h:QPC * h + QPC] = r2.results[c]["yout"]
    return y


# revision 9
# speedup vs baseline: 1.0108x; 1.0108x over previous
"""Trainium2 Bass kernel for nn_PixelCrossAttentionRefiner.

Semantics (matching the reference's torch-style reshape): per batch b, the
flat candidate stream m = k*N + n2 (K=8 slices of N=H*W pixels) is chunked
into groups of 8 consecutive entries; query pixel n attends over entries
8n..8n+7. Key/value entries carry coords (and stage-2 z) of their OWN flat
position n2 = m mod N.

Sharding: 8 cores = 4 batches x 2 halves. Core c: batch b=c//2, half
h=c%2 -> candidate k-slices [4h, 4h+4), queries n in [8192h, 8192h+8192).
Stage-2 keys need z at all N positions of the batch, so z is exchanged on
the host between the two per-batch kernel launches (stage1, stage2).

Layout on device: E-major ([feature, pixel]) tiles; candidate tiles
[128, 512] stack two 512-column ranges of one k-slice (rows 0:64 | 64:128).
Per tile: projections on PE (fp32r / fp16), per-(query,head) scores via an
indicator matmul over partitions, softmax on 8-row tiles, attn expansion
back to [128, 512] via a second indicator matmul, weighted sum + output
projection.
"""

import numpy as np

import concourse.bass as bass
import concourse.mybir as mybir
import concourse.tile as tile
from concourse import bacc
from concourse.bass_utils import run_bass_kernel_spmd

F32 = mybir.dt.float32
F32R = mybir.dt.float32r
F16 = mybir.dt.float16

B, C, H, W = 4, 64, 128, 128
K, E, NH = 8, 64, 4
N = H * W            # 16384 pixels per batch
NCORES = 8
SLICES = 4           # k-slices per core
QPC = N // 2         # queries per core (8192)
TF = 512             # free width per half-tile
TC = 2 * TF          # candidate columns per tile (1024)
NTS = N // TC        # tiles per slice (16)
AX = mybir.AxisListType
ALU = mybir.AluOpType
ACTF = mybir.ActivationFunctionType


# ----------------------------------------------------------------------------
# host-side weight fusion
# ----------------------------------------------------------------------------

def _fuse_weights(inp):
    wq, wk, wv = np.split(inp["in_proj_w"], 3, axis=0)
    bq, bk, bv = np.split(inp["in_proj_b"], 3, axis=0)
    out_w, out_b = inp["out_w"], inp["out_b"]

    Wq1f = inp["q1_w"].T @ wq.T
    bq1f = inp["q1_b"] @ wq.T + bq
    Wk1f = inp["k1_w"].T @ wk.T
    Wv1f = inp["v1_w"].T @ wv.T
    bv1f = inp["v1_b"] @ wv.T + bv
    zb1 = bv1f @ out_w.T + out_b

    Wq2f = inp["q2_w"].T @ wq.T
    bq2f = inp["q2_b"] @ wq.T + bq
    Wk2f = inp["k2_w"].T @ wk.T
    Wv2f = inp["v2_w"].T @ wv.T
    bv2f = inp["v2_b"] @ wv.T + bv
    zb2 = bv2f @ out_w.T + out_b

    def blk(a):  # [64, 64] -> [128, 128] block diagonal
        o = np.zeros((128, 128), a.dtype)
        o[:64, :64] = a
        o[64:, 64:] = a
        return o

    def ablk(rows):  # [4, 64] affine rows -> [8, 128] block diagonal
        o = np.zeros((8, 128), np.float32)
        o[:4, :64] = rows
        o[4:, 64:] = rows
        return o

    z4 = np.zeros((1, E), np.float32)
    wgt = {
        # stage 1
        "wq1": np.concatenate([Wq1f[:64], Wq1f[64:66], bq1f[None], z4], 0).astype(np.float16),
        "wk1x": blk(Wk1f[:64]).astype(np.float32),
        "wk1a": ablk(np.concatenate([Wk1f[64:66], z4, z4], 0)),
        "wv1": blk(Wv1f).astype(np.float32),
        "zb1": np.tile(zb1, 2).astype(np.float32).reshape(128, 1),
        # stage 2
        "wq2": np.concatenate([Wq2f[:64], Wq2f[128:130], bq2f[None], z4], 0).astype(np.float16),
        "wq2z": Wq2f[64:128].astype(np.float16),
        "wk2x": blk(Wk2f[:64]).astype(np.float32),
        "wk2z": blk(Wk2f[64:128]).astype(np.float16),
        "wk2a": ablk(np.concatenate([Wk2f[128:130], z4, z4], 0)),
        "wv2x": blk(Wv2f[:64]).astype(np.float32),
        "wv2z": blk(Wv2f[64:128]).astype(np.float16),
        "zb2": np.tile(zb2, 2).astype(np.float32).reshape(128, 1),
        # shared
        "wo": blk(out_w.T).astype(np.float32),
    }

    # score indicator: partition (64*half + 16h + d) -> col (4*half + h)
    ind_s = np.zeros((128, 8), np.float16)
    for p in range(128):
        ind_s[p, 4 * (p // 64) + (p % 64) // 16] = 1.0
    wgt["ind_s"] = ind_s
    # expansion indicators (4 group variants): variant j maps rows 8j..8j+8
    # (= tile j's (half, head) rows in the group-stacked [32, x] tile) back
    # to partitions 64*half + 16h + d
    ind_er = np.zeros((128, 128), np.float16)
    for j in range(4):
        ind_er[32 * j:32 * j + 8, :] = ind_s.T
    wgt["ind_er"] = ind_er
    return wgt


def _affine_tables():
    i = np.linspace(0.0, 1.0, H, dtype=np.float32)
    j = np.linspace(0.0, 1.0, W, dtype=np.float32)
    ig, jg = np.meshgrid(i, j, indexing="ij")
    aff = np.stack([ig.ravel(), jg.ravel(),
                    np.ones(N, np.float32), np.zeros(N, np.float32)])  # [4, N]
    return aff


# ----------------------------------------------------------------------------
# bass program (one stage)
# ----------------------------------------------------------------------------

WSPECS1 = [("wq1", (68, 64), F16), ("wk1x", (128, 128), F32R),
           ("wk1a", (8, 128), F32R), ("wv1", (128, 128), F32R),
           ("zb1", (128, 1), F32),
           ("ind_s", (128, 8), F16), ("ind_er", (128, 128), F16),
           ("wo", (128, 128), F32)]
WSPECS2 = [("wq2", (68, 64), F16), ("wq2z", (64, 64), F16),
           ("wk2x", (128, 128), F32R), ("wk2z", (128, 128), F16),
           ("wk2a", (8, 128), F32R), ("wv2x", (128, 128), F32R),
           ("wv2z", (128, 128), F16), ("zb2", (128, 1), F32),
           ("ind_s", (128, 8), F16), ("ind_er", (128, 128), F16),
           ("wo", (128, 128), F32)]


def build_program(stage: int):
    nc = bacc.Bacc("TRN2", target_bir_lowering=False, debug=False)

    kc = nc.dram_tensor("kc", [SLICES, C, N], F32R, kind="ExternalInput")
    vc = nc.dram_tensor("vc", [SLICES, C, N], F32R, kind="ExternalInput")
    xq = nc.dram_tensor("xq", [C, QPC], F32, kind="ExternalInput")
    affq = nc.dram_tensor("affq", [4, QPC], F32, kind="ExternalInput")
    affc = nc.dram_tensor("affc", [4, N], F32R, kind="ExternalInput")
    wspecs = WSPECS1 if stage == 1 else WSPECS2
    wd = {nm: nc.dram_tensor(nm, list(sh), dt, kind="ExternalInput")
          for nm, sh, dt in wspecs}
    if stage == 2:
        zfull = nc.dram_tensor("zfull", [C, N], F16, kind="ExternalInput")
        zq = nc.dram_tensor("zq", [C, QPC], F16, kind="ExternalInput")
        out = nc.dram_tensor("yout", [C, QPC], F32, kind="ExternalOutput")
    else:
        out = nc.dram_tensor("zout", [C, QPC], F16, kind="ExternalOutput")

    with tile.TileContext(nc) as tc:
        with (
            tc.tile_pool(name="wpool", bufs=1) as wpool,
            tc.tile_pool(name="io", bufs=3) as io,
            tc.tile_pool(name="zst", bufs=6) as zstp,
            tc.tile_pool(name="aux", bufs=2) as aux,
            tc.tile_pool(name="mid", bufs=2) as mid,
            tc.tile_pool(name="ps_kh", bufs=2, space="PSUM") as ps_kh,
            tc.tile_pool(name="ps_vh", bufs=1, space="PSUM") as ps_vh,
            tc.tile_pool(name="ps_ae", bufs=1, space="PSUM") as ps_ae,
            tc.tile_pool(name="ps_small", bufs=1, space="PSUM") as ps_small,
        ):
            # load weights once
            wt = {}
            for nm, sh, dt in wspecs:
                t = wpool.tile(list(sh), dt, tag=nm)
                nc.sync.dma_start(t[:], wd[nm].ap())
                wt[nm] = t

            for sl in range(SLICES):
                for g in range(NTS // 4):
                    # per-group softmax accumulator rows 8j:8j+8 <- tile j
                    e_all = mid.tile([128, TF], F16, tag="e_all")

                    tiles = []
                    # ---- phase A: per-tile query/key path + exp into e_all
                    for j in range(4):
                        ti = 4 * g + j
                        t0 = ti * TC
                        q0 = sl * (N // K) + ti * (TC // K)

                        kc_t = io.tile([128, TF], F32R, tag="kc_t")
                        nc.sync.dma_start(kc_t[0:64, :], kc[sl, :, t0:t0 + TF])
                        nc.sync.dma_start(kc_t[64:128, :], kc[sl, :, t0 + TF:t0 + TC])
                        aff_t = aux.tile([8, TF], F32R, tag="aff_t")
                        nc.sync.dma_start(aff_t[0:4, :], affc[:, t0:t0 + TF])
                        nc.sync.dma_start(aff_t[4:8, :], affc[:, t0 + TF:t0 + TC])
                        xa_t = aux.tile([68, 128], F32, tag="xa_t")
                        nc.sync.dma_start(xa_t[0:64, :], xq[:, q0:q0 + 128])
                        nc.sync.dma_start(xa_t[64:68, :], affq[:, q0:q0 + 128])
                        xa16 = aux.tile([68, 128], F16, tag="xa16")
                        nc.scalar.copy(xa16[:], xa_t[:])
                        if stage == 2:
                            zq_t = aux.tile([64, 128], F16, tag="zq_t")
                            nc.sync.dma_start(zq_t[:], zq[:, q0:q0 + 128])
                            zst_t = zstp.tile([128, TF], F16, tag="zst_t")
                            nc.sync.dma_start(zst_t[0:64, :], zfull[:, t0:t0 + TF])
                            nc.sync.dma_start(zst_t[64:128, :], zfull[:, t0 + TF:t0 + TC])
                        else:
                            zst_t = None

                        # query projection (two column groups = two query halves)
                        qh_ps = ps_small.tile([128, 64], F32, tag="qh_ps")
                        wq = wt["wq1" if stage == 1 else "wq2"]
                        two = stage == 2
                        nc.tensor.matmul(qh_ps[0:64, :], wq[:], xa16[:, 0:64],
                                         start=True, stop=not two)
                        nc.tensor.matmul(qh_ps[64:128, :], wq[:], xa16[:, 64:128],
                                         start=True, stop=not two)
                        if two:
                            nc.tensor.matmul(qh_ps[0:64, :], wt["wq2z"][:],
                                             zq_t[:, 0:64], start=False, stop=True)
                            nc.tensor.matmul(qh_ps[64:128, :], wt["wq2z"][:],
                                             zq_t[:, 64:128], start=False, stop=True)
                        qh16 = mid.tile([128, 64], F16, tag="qh16")
                        nc.vector.tensor_copy(qh16[:], qh_ps[:])

                        # key projection
                        kh_ps = ps_kh.tile([128, TF], F32, tag="kh_ps")
                        if stage == 1:
                            nc.tensor.matmul(kh_ps[:], wt["wk1x"][:],
                                             kc_t[:], start=True, stop=False)
                            nc.tensor.matmul(kh_ps[:], wt["wk1a"][:],
                                             aff_t[:], start=False, stop=True)
                        else:
                            nc.tensor.matmul(kh_ps[:], wt["wk2x"][:],
                                             kc_t[:], start=True, stop=False)
                            nc.tensor.matmul(kh_ps[:], wt["wk2a"][:],
                                             aff_t[:], start=False, stop=False)
                            nc.tensor.matmul(kh_ps[:], wt["wk2z"][:], zst_t[:],
                                             start=False, stop=True)

                        # prods (kh read straight from PSUM) -> scores -> exp
                        p16 = mid.tile([128, TF], F16, tag="p16")
                        nc.vector.tensor_mul(
                            p16[:].rearrange("p (q k) -> p q k", k=K),
                            kh_ps[:].rearrange("p (q k) -> p q k", k=K),
                            qh16[:].broadcast_to([128, 64, K]))
                        sc_ps = ps_small.tile([8, TF], F32, tag="sc_ps")
                        nc.tensor.matmul(sc_ps[:], wt["ind_s"][:], p16[:],
                                         start=True, stop=True)
                        nc.scalar.activation(e_all[32 * j:32 * j + 8, :], sc_ps[:],
                                             ACTF.Exp, scale=0.25)
                        tiles.append((q0, zst_t))

                    # ---- phase B: batched denominator + reciprocal
                    den = aux.tile([128, 64], F32, tag="den")
                    nc.vector.tensor_reduce(
                        den[:], e_all[:].rearrange("p (q k) -> p q k", k=K),
                        axis=AX.X, op=ALU.add)
                    rcp = aux.tile([128, 64], F16, tag="rcp")
                    with nc.allow_low_precision(reason="softmax reciprocal in f16"):
                        nc.vector.reciprocal(rcp[:], den[:])

                    # ---- phase C: per-tile value path, weighted sum, out proj
                    for j in range(4):
                        ti = 4 * g + j
                        t0 = ti * TC
                        q0, zst_t = tiles[j]

                        vc_t = io.tile([128, TF], F32R, tag="vc_t")
                        nc.sync.dma_start(vc_t[0:64, :], vc[sl, :, t0:t0 + TF])
                        nc.sync.dma_start(vc_t[64:128, :], vc[sl, :, t0 + TF:t0 + TC])
                        vh_ps = ps_vh.tile([128, TF], F32, tag="vh_ps")
                        if stage == 1:
                            nc.tensor.matmul(vh_ps[:], wt["wv1"][:],
                                             vc_t[:], start=True, stop=True)
                        else:
                            nc.tensor.matmul(vh_ps[:], wt["wv2x"][:],
                                             vc_t[:], start=True, stop=False)
                            nc.tensor.matmul(vh_ps[:], wt["wv2z"][:], zst_t[:],
                                             start=False, stop=True)

                        # expand unnormalized exp for this tile
                        ae_ps = ps_ae.tile([128, TF], F32, tag="ae_ps")
                        nc.tensor.matmul(ae_ps[:], wt["ind_er"][32 * j:32 * j + 8, :],
                                         e_all[32 * j:32 * j + 8, :],
                                         start=True, stop=True,
                                         tile_position=(32 * j, 0))
                        ae16 = mid.tile([128, TF], F16, tag="ae16")
                        nc.scalar.copy(ae16[:], ae_ps[:])

                        w16 = mid.tile([128, TF], F16, tag="w16")
                        nc.vector.tensor_mul(w16[:], vh_ps[:], ae16[:])
                        o_t = mid.tile([128, 64], F32, tag="o_t")
                        nc.vector.tensor_reduce(
                            o_t[:], w16[:].rearrange("p (q k) -> p q k", k=K),
                            axis=AX.X, op=ALU.add)

                        # normalize o by the expanded reciprocal denominator
                        rx_ps = ps_small.tile([128, 64], F32, tag="rx_ps")
                        nc.tensor.matmul(rx_ps[:], wt["ind_er"][32 * j:32 * j + 8, :],
                                         rcp[32 * j:32 * j + 8, :],
                                         start=True, stop=True,
                                         tile_position=(32 * j, 0))
                        on_t = mid.tile([128, 64], F32, tag="on_t")
                        nc.vector.tensor_mul(on_t[:], o_t[:], rx_ps[:])

                        z_ps = ps_small.tile([128, 64], F32, tag="z_ps")
                        nc.tensor.matmul(z_ps[:], wt["wo"][:], on_t[:],
                                         start=True, stop=True)
                        zb = wt["zb1" if stage == 1 else "zb2"]
                        odt = F16 if stage == 1 else F32
                        z_sb = mid.tile([128, 64], odt, tag="z_sb")
                        nc.scalar.activation(z_sb[:], z_ps[:], ACTF.Identity,
                                             bias=zb[:, 0:1])
                        nc.sync.dma_start(out.ap()[:, q0:q0 + 64], z_sb[0:64, :])
                        nc.sync.dma_start(out.ap()[:, q0 + 64:q0 + 128], z_sb[64:128, :])

    nc.compile()
    return nc


# ----------------------------------------------------------------------------
# host orchestration
# ----------------------------------------------------------------------------

_progs = {}


def _prog(stage):
    if stage not in _progs:
        _progs[stage] = build_program(stage)
    return _progs[stage]


def kernel(**inputs):
    inputs = {k: np.ascontiguousarray(np.asarray(v)) for k, v in inputs.items()}
    wgt = _fuse_weights(inputs)
    aff = _affine_tables()

    hfc = inputs["HF_cands"]   # [B, K, C, H, W]
    zc = inputs["Z_cands"]
    hfs = inputs["HF_star"]    # [B, C, H, W]

    wmap1 = {nm: wgt[nm] for nm, _, _ in WSPECS1}
    wmap2 = {nm: wgt[nm] for nm, _, _ in WSPECS2}

    def core_base(c):
        b, h = divmod(c, 2)
        sl = slice(4 * h, 4 * h + 4)
        qs = slice(QPC * h, QPC * h + QPC)
        return {
            "kc": hfc[b, sl].reshape(SLICES, C, N),
            "vc": zc[b, sl].reshape(SLICES, C, N),
            "xq": hfs[b].reshape(C, N)[:, qs],
            "affq": aff[:, qs],
            "affc": aff,
        }

    bases = [core_base(c) for c in range(NCORES)]

    in1 = [{**bases[c], **wmap1} for c in range(NCORES)]
    r1 = run_bass_kernel_spmd(_prog(1), in1, core_ids=list(range(NCORES)))
    z_half = [r1.results[c]["zout"] for c in range(NCORES)]  # [64, 8192] f16

    in2 = []
    for c in range(NCORES):
        b, h = divmod(c, 2)
        zf = np.concatenate([z_half[2 * b], z_half[2 * b + 1]], axis=1)  # [64, N]
        in2.append({**bases[c], **wmap2, "zfull": zf, "zq": z_half[c]})
    r2 = run_bass_kernel_spmd(_prog(2), in2, core_ids=list(range(NCORES)))

    y = np.empty((B, C, H, W), np.float32)
    for c in range(NCORES):
        b, h = divmod(c, 2)
        y[b].reshape(C, N)[:, QPC * h:QPC * h + QPC] = r2.results[c]["yout"]
    return y
